# revision 1
# baseline (speedup 1.0000x reference)
"""ClassConditionalLM log-likelihood kernel for 8 Trainium2 NeuronCores.

Math:
  out[n] = logsumexp_j( prior'_j - S'[j,n] + corr[j,n] )
where
  S'[j,n]  = sum_l maskf[l,n] * ((z_acc+acc)[l,j] - prop[l] + log(K-1))
  corr[j,n]= sum_l [votes[l,n] == j+1] * (2*acc[l,j] + log(K-1))
  prior'_j = class_prior_j - sum_l logaddexp(prop[l], 0)

Device strategy (per core, data-parallel over instances; ~450us/core per the
cost-model timeline, within ~10% of the PE mask-stream floor):
  - votes^T bf16 [L=128, n] streamed in chunks of F=2048.
  - per vote symbol v: a one-hot mask (votes^T == v) feeds a tiny
    block-diagonal weight matmul accumulated in PSUM rows 0..63 (corr^T).
    The first NPAIR symbol pairs use fp8 DoubleRow matmuls (256-deep
    contraction, 2x PE rate; fp8 hi/lo weight split beats bf16 precision);
    the rest are bf16 masks built on DVE (4x mode) and GPSIMD (GPS of them).
  - S' is folded into the same PSUM rows with NEGATED hi/lo bf16 weights
    (rhs = maskf = votes!=0), so PSUM = corr - S' directly; no extra drain.
  - Tail: PE transposes 128-column tiles of D^T = PSUM + prior'; DVE does one
    batched max-reduce per chunk; ACT does exp with accumulated row-sum (only
    Exp runs during the loop so its table loads once). All ln's happen in one
    pass at the very end, followed by a single strided output DMA.
"""

import math

import numpy as np
import ml_dtypes

N, L, K = 131072, 128, 64
M = 8                    # NeuronCores
NC_N = N // M            # 16384 instances per core
F = 2048                 # instances per chunk
SUB = 512                # matmul free-dim subtile (one PSUM bank)
TPT = F // 128           # transpose tiles per chunk
BLK = 32                 # corr lhsT block width (PE tile col granularity)
NPAIR = 12               # pairs with fp8 masks (DoubleRow matmuls on PE)
GP_PAIRS = 2             # of those, pairs whose fp8 masks GPSIMD writes
ACT_PAIRS = 0            # extra pairs: bf16 masks on DVE, cast to fp8 on ACT
GPS = 6                  # bf16 symbol masks built on GPSIMD instead of DVE
LOGKM1 = math.log(K - 1)

_BASS_CACHE: dict = {}


def _build_bass(nc_n: int):
    import concourse.mybir as mybir
    from concourse.bacc import Bacc
    from concourse.tile import TileContext
    from concourse.masks import make_identity

    dt = mybir.dt
    Alu = mybir.AluOpType
    Act = mybir.ActivationFunctionType

    nchunk = nc_n // F
    assert nchunk * F == nc_n
    ncols = nchunk * TPT         # total 128-instance column tiles

    nc = Bacc()
    votest = nc.dram_tensor("votest", [L, nc_n], dt.bfloat16, kind="ExternalInput")
    wblk = nc.dram_tensor("wblk", [L, K * BLK], dt.bfloat16, kind="ExternalInput")
    wph = nc.dram_tensor("wph", [L, max(NPAIR + ACT_PAIRS, 1) * 2 * BLK], dt.float8e4,
                         kind="ExternalInput")
    wpl = nc.dram_tensor("wpl", [L, max(NPAIR + ACT_PAIRS, 1) * 2 * BLK], dt.float8e4,
                         kind="ExternalInput")
    nshi = nc.dram_tensor("nshi", [L, K], dt.bfloat16, kind="ExternalInput")
    nslo = nc.dram_tensor("nslo", [L, K], dt.bfloat16, kind="ExternalInput")
    prior = nc.dram_tensor("prior", [K, 1], dt.float32, kind="ExternalInput")
    out = nc.dram_tensor("out", [nc_n], dt.float32, kind="ExternalOutput")

    with TileContext(nc) as tc:
        with (
            tc.tile_pool(name="const", bufs=1) as cpool,
            tc.tile_pool(name="vt", bufs=3) as vpool,
            tc.tile_pool(name="mask", bufs=8) as mpool,
            tc.tile_pool(name="work", bufs=2) as wpool,
            tc.tile_pool(name="tail", bufs=6) as tpool,
            tc.tile_pool(name="pc", bufs=1, space="PSUM") as pcpool,
            tc.tile_pool(name="pt", bufs=2, space="PSUM") as ptpool,
        ):
            ident = cpool.tile([128, 128], dt.float32, tag="ident")
            make_identity(nc, ident[:])
            wblk_sb = cpool.tile([L, K * BLK], dt.bfloat16, tag="wblk")
            nc.sync.dma_start(out=wblk_sb[:], in_=wblk[:, :])
            wph_sb = cpool.tile([L, max(NPAIR + ACT_PAIRS, 1) * 2 * BLK], dt.float8e4, tag="wph")
            nc.sync.dma_start(out=wph_sb[:], in_=wph[:, :])
            wpl_sb = cpool.tile([L, max(NPAIR + ACT_PAIRS, 1) * 2 * BLK], dt.float8e4, tag="wpl")
            nc.sync.dma_start(out=wpl_sb[:], in_=wpl[:, :])
            shi_sb = cpool.tile([L, K], dt.bfloat16, tag="shi")
            nc.sync.dma_start(out=shi_sb[:], in_=nshi[:, :])
            slo_sb = cpool.tile([L, K], dt.bfloat16, tag="slo")
            nc.sync.dma_start(out=slo_sb[:], in_=nslo[:, :])
            prior_sb = cpool.tile([K, 1], dt.float32, tag="prior")
            nc.sync.dma_start(out=prior_sb[:], in_=prior[:, :])
            # per-column-tile logsumexp pieces, stashed until the end
            ssum_all = cpool.tile([128, ncols], dt.float32, tag="ssum_all")
            mneg_all = cpool.tile([128, ncols], dt.float32, tag="mneg_all")

            for c in range(nchunk):
                vt = vpool.tile([L, F], dt.bfloat16, tag="vt")
                nc.sync.dma_start(out=vt[:], in_=votest[:, c * F:(c + 1) * F])

                pc = pcpool.tile([64, F], dt.float32, tag="pc")

                # -S' into PSUM rows 0..63 (negated hi/lo bf16 weights)
                maskf = wpool.tile([L, F], dt.bfloat16, tag="maskf")
                nc.vector.tensor_scalar(
                    out=maskf[:], in0=vt[:], scalar1=0.0, scalar2=None,
                    op0=Alu.not_equal,
                )
                for s in range(F // SUB):
                    sl = slice(s * SUB, (s + 1) * SUB)
                    nc.tensor.matmul(
                        out=pc[:, sl], lhsT=shi_sb[:], rhs=maskf[:, sl],
                        start=True, stop=False, skip_group_check=True,
                    )
                    nc.tensor.matmul(
                        out=pc[:, sl], lhsT=slo_sb[:], rhs=maskf[:, sl],
                        start=False, stop=False, skip_group_check=True,
                    )

                # corr accumulated on top, in two 32-row windows.
                # First NPAIR symbol pairs go through fp8 DoubleRow matmuls
                # (256-deep contraction, 2x PE rate; hi/lo fp8 weight split
                # keeps precision better than bf16).
                for p in range(NPAIR + ACT_PAIRS):
                    v1 = 2 * p + 1
                    q = ((v1 - 1) // BLK) * BLK
                    mp = mpool.tile([L, 2 * F], dt.float8e4, tag="maskp")
                    if p < NPAIR:
                        # DVE (or GPSIMD) writes the fp8 pair-mask directly
                        meng = nc.gpsimd if p < GP_PAIRS else nc.vector
                        meng.tensor_scalar(
                            out=mp[:, 0:F], in0=vt[:], scalar1=float(v1),
                            scalar2=None, op0=Alu.is_equal,
                        )
                        meng.tensor_scalar(
                            out=mp[:, F:2 * F], in0=vt[:], scalar1=float(v1 + 1),
                            scalar2=None, op0=Alu.is_equal,
                        )
                    else:
                        # bf16 masks at DVE 4x rate, then one wide ACT cast
                        mpb = mpool.tile([L, 2 * F], dt.bfloat16, tag="maskpb")
                        nc.vector.tensor_scalar(
                            out=mpb[:, 0:F], in0=vt[:], scalar1=float(v1),
                            scalar2=None, op0=Alu.is_equal,
                        )
                        nc.vector.tensor_scalar(
                            out=mpb[:, F:2 * F], in0=vt[:], scalar1=float(v1 + 1),
                            scalar2=None, op0=Alu.is_equal,
                        )
                        nc.scalar.copy(out=mp[:], in_=mpb[:])
                    mp3 = mp[:].rearrange("l (i f) -> l i f", i=2)
                    for s in range(F // SUB):
                        for wsb in (wph_sb, wpl_sb):
                            nc.tensor.matmul(
                                out=pc[q:q + BLK, s * SUB:(s + 1) * SUB],
                                lhsT=wsb[:, p * 2 * BLK:(p + 1) * 2 * BLK]
                                .rearrange("l (i m) -> l i m", i=2),
                                rhs=mp3[:, :, s * SUB:(s + 1) * SUB],
                                start=False, stop=False,
                                perf_mode=mybir.MatmulPerfMode.DoubleRow,
                                skip_group_check=True,
                            )

                # remaining symbols in bf16; some masks built on the
                # (otherwise idle) GPSIMD engine to relieve the DVE.
                rest = list(range(2 * (NPAIR + ACT_PAIRS) + 1, K + 1))
                gp_every = max(1, len(rest) // max(GPS, 1))
                for i, v in enumerate(rest):
                    q = ((v - 1) // BLK) * BLK
                    mk = mpool.tile([L, F], dt.bfloat16, tag="mask")
                    on_gp = (i % gp_every == gp_every - 1) and (GPS > 0)
                    eng = nc.gpsimd if on_gp else nc.vector
                    eng.tensor_scalar(
                        out=mk[:], in0=vt[:], scalar1=float(v), scalar2=None,
                        op0=Alu.is_equal,
                    )
                    for s in range(F // SUB):
                        sl = slice(s * SUB, (s + 1) * SUB)
                        nc.tensor.matmul(
                            out=pc[q:q + BLK, sl],
                            lhsT=wblk_sb[:, (v - 1) * BLK:v * BLK],
                            rhs=mk[:, sl],
                            start=False, stop=(v == K),
                            skip_group_check=True,
                        )

                # D^T = PSUM + prior'  [64, F] fp32 in SBUF (on ACT: frees DVE)
                dT = wpool.tile([64, F], dt.float32, tag="dT")
                nc.scalar.activation(
                    out=dT[:], in_=pc[:, :], func=Act.Identity,
                    bias=prior_sb[:, 0:1], scale=1.0,
                )

                # tail: transpose 128-column tiles into one wide PSUM tile,
                # one batched max-reduce, then per-tile exp with accum-sum
                ptw = ptpool.tile([128, TPT * K], dt.float32, tag="ptw")
                for t in range(TPT):
                    nc.tensor.transpose(
                        out=ptw[:, t * K:(t + 1) * K],
                        in_=dT[:, t * 128:(t + 1) * 128],
                        identity=ident[0:64, 0:64],
                    )
                cols = slice(c * TPT, (c + 1) * TPT)
                nc.vector.tensor_reduce(
                    out=mneg_all[:, cols],
                    in_=ptw[:].rearrange("p (t k) -> p t k", k=K),
                    axis=mybir.AxisListType.X, op=Alu.max, negate=True,
                )
                for t in range(TPT):
                    col = c * TPT + t
                    escr = tpool.tile([128, K], dt.float32, tag="escr")
                    nc.scalar.activation(
                        out=escr[:], in_=ptw[:, t * K:(t + 1) * K], func=Act.Exp,
                        bias=mneg_all[:, col:col + 1], scale=1.0,
                        accum_out=ssum_all[:, col:col + 1],
                    )

            # finale: ln over all stashed sums, add back maxes, single DMA out
            lns = cpool.tile([128, ncols], dt.float32, tag="lns")
            nc.scalar.activation(out=lns[:], in_=ssum_all[:], func=Act.Ln)
            outT = cpool.tile([128, ncols], dt.float32, tag="outT")
            nc.vector.tensor_tensor(
                out=outT[:], in0=lns[:], in1=mneg_all[:], op=Alu.subtract,
            )
            oview = out[:].rearrange("(x p) -> p x", p=128)
            nc.sync.dma_start(out=oview, in_=outT[:])
    nc.finalize()
    return nc


def _get_bass(nc_n: int):
    if nc_n not in _BASS_CACHE:
        _BASS_CACHE[nc_n] = _build_bass(nc_n)
    return _BASS_CACHE[nc_n]


def _prepare_host(votes, accuracy, propensity, class_balance):
    bf16 = ml_dtypes.bfloat16
    votes = np.asarray(votes)
    accuracy = np.asarray(accuracy, dtype=np.float32)
    propensity = np.asarray(propensity, dtype=np.float32)
    class_balance = np.asarray(class_balance, dtype=np.float32)

    # values 0..64 are exact in bf16
    votesT = np.ascontiguousarray(votes.T.astype(np.float32).astype(bf16))

    z_acc = np.logaddexp(accuracy, -accuracy)
    stab = (z_acc + accuracy - propensity[:, None] + LOGKM1).astype(np.float32)
    shi = stab.astype(bf16)
    slo = (stab - shi.astype(np.float32)).astype(bf16)
    nshi = np.ascontiguousarray(-shi)       # negated: PSUM accumulates -S'
    nslo = np.ascontiguousarray(-slo)

    w = 2.0 * accuracy + LOGKM1                      # [L, K]
    wblk = np.zeros((L, K, BLK), np.float32)
    jj = np.arange(K)
    wblk[:, jj, jj % BLK] = w                        # 32-wide block columns
    wblk = np.ascontiguousarray(wblk.reshape(L, K * BLK).astype(bf16))

    # fp8 DoubleRow pair weights, hi/lo split
    f8 = ml_dtypes.float8_e4m3
    npair = max(NPAIR + ACT_PAIRS, 1)
    wph = np.zeros((L, npair, 2, BLK), np.float32)
    wpl = np.zeros((L, npair, 2, BLK), np.float32)
    w_hi = w.astype(f8).astype(np.float32)
    w_lo = (w - w_hi).astype(f8).astype(np.float32)
    for p in range(NPAIR + ACT_PAIRS):
        for i in range(2):
            j = 2 * p + i                            # target class row
            wph[:, p, i, j % BLK] = w_hi[:, j]
            wpl[:, p, i, j % BLK] = w_lo[:, j]
    wph = np.ascontiguousarray(wph.reshape(L, npair * 2 * BLK).astype(f8))
    wpl = np.ascontiguousarray(wpl.reshape(L, npair * 2 * BLK).astype(f8))

    zprop = np.logaddexp(propensity, 0.0)
    cbm = class_balance.max()
    cb = class_balance - (np.log(np.sum(np.exp(class_balance - cbm))) + cbm)
    priorp = np.ascontiguousarray(
        (cb - zprop.sum()).astype(np.float32).reshape(K, 1)
    )
    return votesT, wblk, wph, wpl, nshi, nslo, priorp


def _run(votes, accuracy, propensity, class_balance, trace=False):
    from concourse.bass_utils import run_bass_kernel_spmd

    votesT, wblk, wph, wpl, nshi, nslo, priorp = _prepare_host(
        votes, accuracy, propensity, class_balance
    )
    nc = _get_bass(NC_N)
    in_maps = []
    for c in range(M):
        in_maps.append({
            "votest": np.ascontiguousarray(votesT[:, c * NC_N:(c + 1) * NC_N]),
            "wblk": wblk,
            "wph": wph,
            "wpl": wpl,
            "nshi": nshi,
            "nslo": nslo,
            "prior": priorp,
        })
    res = run_bass_kernel_spmd(
        nc, in_maps, core_ids=list(range(M)), trace=trace
    )
    out = np.concatenate([r["out"] for r in res.results])
    return out.astype(np.float32), res


def kernel(votes, accuracy, propensity, class_balance):
    out, _ = _run(votes, accuracy, propensity, class_balance)
    return out


def kernel_with_stats(votes, accuracy, propensity, class_balance):
    try:
        out, res = _run(votes, accuracy, propensity, class_balance, trace=True)
    except (ImportError, ModuleNotFoundError):
        # no NTFF profiling hook in this environment; run without trace
        out, res = _run(votes, accuracy, propensity, class_balance, trace=False)
    return out, res


def simulate_ns() -> float:
    """Cost-model timeline estimate (ns) of one core's NEFF execution."""
    from concourse.timeline_sim import TimelineSim

    return TimelineSim(_get_bass(NC_N), trace=False).simulate()



# revision 40
# speedup vs baseline: 1.5116x; 1.5116x over previous
"""ClassConditionalLM log-likelihood kernel for 8 Trainium2 NeuronCores.

Math:
  out[n] = logsumexp_j( prior'_j - S'[j,n] + corr[j,n] )
where
  S'[j,n]  = sum_l maskf[l,n] * ((z_acc+acc)[l,j] - prop[l] + log(K-1))
  corr[j,n]= sum_l [votes[l,n] == j+1] * (2*acc[l,j] + log(K-1))
  prior'_j = class_prior_j - sum_l logaddexp(prop[l], 0)

Device strategy (per core, data-parallel over instances). The kernel is
mask-supply bound: 64 one-hot symbol masks per chunk must be produced on
the elementwise engines and consumed by PE matmuls. v2 balances all four
engines:
  - vt2 = [vt ; vt-1] per chunk, so ONE is_equal(v) op yields the fp8 mask
    PAIR for symbols (v, v+1); pair masks feed single-fp8 DoubleRow
    matmuls (256-deep, 0.5 cy/row).
  - pairs are split across DVE (is_equal), GPSIMD (is_equal), and ACT
    (tent = Relu(1-Abs(vt2-v)), 2 ops/pair, same natural_log_exp table).
  - remaining symbols use bf16 masks on DVE (4x mode) + bf16 matmuls.
  - S' folds into the same PSUM with negated hi/lo bf16 weights.
  - tail: dT = bf16(PSUM + (prior'-C)) recentred by a host-computed C so
    bf16 is accurate; PE transposes at 1cy/row; one batched DVE max, one
    batched DVE (x-max) diff, ONE batched ACT exp per chunk, one batched
    DVE sum. All ln's + add-back-C happen once at the end.
"""

import math

import numpy as np
import ml_dtypes

N, L, K = 131072, 128, 64
M = 8                    # NeuronCores
NC_N = N // M            # 16384 instances per core
F = 2048                 # instances per chunk
SUB = 512                # matmul free-dim subtile (one PSUM bank)
TPT = F // 128           # transpose tiles per chunk
BLK = 64                 # corr lhsT block width (DR dst must start at 0)
NPAIR_DVE = 0            # symbol pairs masked on DVE (fp8 is_equal)
NPAIR_GPS = 5            # symbol pairs masked on GPSIMD (fp8 is_equal)
NPAIR_ACT = 4            # symbol pairs masked on ACT (fp8 tent)
NPAIR = NPAIR_DVE + NPAIR_GPS + NPAIR_ACT
NSING = K - 2 * NPAIR    # single symbols: bf16 X-masks on DVE, bitcast DR
NGPS_SING = 1            # of those, X-mask singles built on GPSIMD
LOGKM1 = math.log(K - 1)
# bf16 value whose two bytes are each the fp8e4m3 value 1.875: a mask
# emitted as X*[v==c] in bf16, bitcast to fp8 pairs, feeds a DoubleRow
# matmul where both byte-rows carry the mask scaled by 1.875.
XMASK = 0.74609375       # bf16 0x3F3F
XBYTE = 1.875            # fp8e4m3 0x3F

_BASS_CACHE: dict = {}


def _build_bass(nc_n: int):
    import concourse.mybir as mybir
    from concourse.bacc import Bacc
    from concourse.tile import TileContext
    from concourse.masks import make_identity

    dt = mybir.dt
    Alu = mybir.AluOpType
    Act = mybir.ActivationFunctionType

    nchunk = nc_n // F
    assert nchunk * F == nc_n
    ncols = nchunk * TPT         # total 128-instance column tiles

    nc = Bacc()
    votest = nc.dram_tensor("votest", [L, nc_n], dt.bfloat16, kind="ExternalInput")
    wph = nc.dram_tensor("wph", [L, max(NPAIR, 1) * 2 * BLK], dt.float8e4,
                         kind="ExternalInput")
    ws8 = nc.dram_tensor("ws8", [L, NSING * 2 * BLK], dt.float8e4,
                         kind="ExternalInput")
    abias = nc.dram_tensor("abias", [L, max(NPAIR_ACT, 1)], dt.float32,
                           kind="ExternalInput")
    nshi = nc.dram_tensor("nshi", [L, K], dt.bfloat16, kind="ExternalInput")
    nslo = nc.dram_tensor("nslo", [L, K], dt.bfloat16, kind="ExternalInput")
    prior = nc.dram_tensor("prior", [K, 1], dt.float32, kind="ExternalInput")
    out = nc.dram_tensor("out", [nc_n], dt.float32, kind="ExternalOutput")

    with TileContext(nc) as tc:
        with (
            tc.tile_pool(name="const", bufs=1) as cpool,
            tc.tile_pool(name="vt", bufs=3) as vpool,
            tc.tile_pool(name="smask", bufs=8) as spool,
            tc.tile_pool(name="dmask", bufs=3) as dpool,
            tc.tile_pool(name="gmask", bufs=5) as gpool,
            tc.tile_pool(name="amask", bufs=2) as apool,
            tc.tile_pool(name="work", bufs=2) as wpool,
            tc.tile_pool(name="tail", bufs=3) as tpool,
            tc.tile_pool(name="pc", bufs=1, space="PSUM") as pcpool,
            tc.tile_pool(name="pt", bufs=2, space="PSUM") as ptpool,
        ):
            ident = cpool.tile([128, 128], dt.float32, tag="ident")
            make_identity(nc, ident[:])
            identb = cpool.tile([128, 128], dt.bfloat16, tag="identb")
            nc.vector.tensor_copy(out=identb[:], in_=ident[:])
            wph_sb = cpool.tile([L, max(NPAIR, 1) * 2 * BLK], dt.float8e4, tag="wph")
            nc.sync.dma_start(out=wph_sb[:], in_=wph[:, :])
            ws8_sb = cpool.tile([L, NSING * 2 * BLK], dt.float8e4, tag="ws8")
            nc.sync.dma_start(out=ws8_sb[:], in_=ws8[:, :])
            abias_sb = cpool.tile([L, max(NPAIR_ACT, 1)], dt.float32, tag="abias")
            nc.sync.dma_start(out=abias_sb[:], in_=abias[:, :])
            shi_sb = cpool.tile([L, K], dt.bfloat16, tag="shi")
            nc.sync.dma_start(out=shi_sb[:], in_=nshi[:, :])
            slo_sb = cpool.tile([L, K], dt.bfloat16, tag="slo")
            nc.sync.dma_start(out=slo_sb[:], in_=nslo[:, :])
            prior_sb = cpool.tile([K, 1], dt.float32, tag="prior")
            nc.sync.dma_start(out=prior_sb[:], in_=prior[:, :])
            # per-column-tile logsumexp pieces, stashed until the end
            ssum_all = cpool.tile([128, ncols], dt.float32, tag="ssum_all")
            mneg_all = cpool.tile([128, ncols], dt.float32, tag="mneg_all")

            # Estimated engine-busy durations (ns) used only to choose a good
            # static issue order; the hardware semaphores enforce correctness.
            EST = {
                "sub": 654.0, "maskf": 654.0, "dve_single": 654.0,
                "dve_pair": 2254.0, "gps_pair": 5784.0, "gps_xsingle": 2939.0,
                "act_pair": 7566.0, "act_dT": 1994.0, "act_exp": 1223.0,
                "dve_max": 783.0, "dve_diff": 783.0, "dve_sum": 783.0,
                "pe_pass_bf16": 853.0, "pe_pass_dr": 427.0,
            }
            clk = {"dve": 0.0, "gps": 0.0, "act": 0.0}
            prev_tail = None  # deferred tail emitters of the previous chunk

            def tick(eng, key):
                clk[eng] += EST[key]
                return clk[eng]

            for c in range(nchunk):
                vt2 = vpool.tile([L, 2 * F], dt.bfloat16, tag="vt2")
                nc.sync.dma_start(out=vt2[:, 0:F], in_=votest[:, c * F:(c + 1) * F])
                nc.vector.tensor_scalar(
                    out=vt2[:, F:2 * F], in0=vt2[:, 0:F], scalar1=1.0,
                    scalar2=None, op0=Alu.subtract,
                )
                tick("dve", "sub")

                pc = pcpool.tile([64, F], dt.float32, tag="pc")

                maskf = wpool.tile([L, F], dt.bfloat16, tag="maskf")
                nc.vector.tensor_scalar(
                    out=maskf[:], in0=vt2[:, 0:F], scalar1=0.0, scalar2=None,
                    op0=Alu.not_equal,
                )
                maskf_ready = tick("dve", "maskf")

                # previous chunk's tail: GPS diff first (frees its ptw), then
                # the DVE max/sum — emitted here so no engine stalls on them.
                if prev_tail is not None:
                    prev_tail()
                    prev_tail = None

                # ---- mask production streams (fixed per-engine order) ----
                jobs = []  # (ready_est, pass_cost_key, emit_matmul_fn)

                def dr_pass(mp, p, q, stop=False):
                    mp3 = mp[:].rearrange("l (i f) -> l i f", i=2)
                    lh = (wph_sb[:, p * 2 * BLK:(p + 1) * 2 * BLK]
                          .rearrange("l (i m) -> l i m", i=2))

                    def emit(stop=stop):
                        for s in range(F // SUB):
                            nc.tensor.matmul(
                                out=pc[q:q + BLK, s * SUB:(s + 1) * SUB],
                                lhsT=lh,
                                rhs=mp3[:, :, s * SUB:(s + 1) * SUB],
                                start=False,
                                stop=stop and (s == F // SUB - 1),
                                perf_mode=mybir.MatmulPerfMode.DoubleRow,
                                skip_group_check=True,
                            )
                    return emit

                def sg_pass(mk, si, q, stop=False):
                    # bf16 X-mask bitcast to fp8: both bytes carry 1.875*m,
                    # DoubleRow contracts byte-rows with hi/lo fp8 weights.
                    mk8 = (mk[:].bitcast(dt.float8e4)
                           .rearrange("l (f i) -> l i f", i=2))
                    lh = (ws8_sb[:, si * 2 * BLK:(si + 1) * 2 * BLK]
                          .rearrange("l (i m) -> l i m", i=2))

                    def emit(stop=stop):
                        for s in range(F // SUB):
                            nc.tensor.matmul(
                                out=pc[q:q + BLK, s * SUB:(s + 1) * SUB],
                                lhsT=lh,
                                rhs=mk8[:, :, s * SUB:(s + 1) * SUB],
                                start=False,
                                stop=stop and (s == F // SUB - 1),
                                perf_mode=mybir.MatmulPerfMode.DoubleRow,
                                skip_group_check=True,
                            )
                    return emit

                # DVE: singles with fp8 pairs spread between them; the last
                # NGPS_SING singles go to GPSIMD instead.
                all_singles = list(range(2 * NPAIR + 1, K + 1))
                gps_singles = all_singles[len(all_singles) - NGPS_SING:] \
                    if NGPS_SING else []
                singles = all_singles[:len(all_singles) - NGPS_SING]
                dve_pairs = list(range(NPAIR_DVE))
                n_sl = len(singles)
                dve_stream = []
                pi = 0
                for i, v in enumerate(singles):
                    if pi < len(dve_pairs) and i * (len(dve_pairs) + 1) >= (pi + 1) * n_sl:
                        dve_stream.append(("pair", dve_pairs[pi]))
                        pi += 1
                    dve_stream.append(("single", v))
                while pi < len(dve_pairs):
                    dve_stream.append(("pair", dve_pairs[pi]))
                    pi += 1
                for kind, x in dve_stream:
                    if kind == "single":
                        v = x
                        si = v - (2 * NPAIR + 1)
                        q = ((v - 1) // BLK) * BLK
                        mk = spool.tile([L, F], dt.bfloat16, tag="mask")
                        nc.vector.tensor_scalar(
                            out=mk[:], in0=vt2[:, 0:F], scalar1=float(v),
                            scalar2=XMASK, op0=Alu.is_equal, op1=Alu.mult,
                        )
                        jobs.append((tick("dve", "dve_single"), "pe_pass_dr",
                                     sg_pass(mk, si, q)))
                    else:
                        p = x
                        v1 = 2 * p + 1
                        q = ((v1 - 1) // BLK) * BLK
                        mp = dpool.tile([L, 2 * F], dt.float8e4, tag="maskp")
                        nc.vector.tensor_scalar(
                            out=mp[:], in0=vt2[:], scalar1=float(v1),
                            scalar2=None, op0=Alu.is_equal,
                        )
                        jobs.append((tick("dve", "dve_pair"), "pe_pass_dr",
                                     dr_pass(mp, p, q)))

                # GPSIMD: fp8 pair masks, then NGPS_SING X-mask singles
                for p in range(NPAIR_DVE, NPAIR_DVE + NPAIR_GPS):
                    v1 = 2 * p + 1
                    q = ((v1 - 1) // BLK) * BLK
                    mp = gpool.tile([L, 2 * F], dt.float8e4, tag="maskpg")
                    nc.gpsimd.tensor_scalar(
                        out=mp[:], in0=vt2[:], scalar1=float(v1),
                        scalar2=None, op0=Alu.is_equal,
                    )
                    jobs.append((tick("gps", "gps_pair"), "pe_pass_dr",
                                 dr_pass(mp, p, q)))
                for v in gps_singles:
                    si = v - (2 * NPAIR + 1)
                    q = ((v - 1) // BLK) * BLK
                    mk = gpool.tile([L, F], dt.bfloat16, tag="maskxg")
                    nc.gpsimd.tensor_scalar(
                        out=mk[:], in0=vt2[:, 0:F], scalar1=float(v),
                        scalar2=XMASK, op0=Alu.is_equal, op1=Alu.mult,
                    )
                    jobs.append((tick("gps", "gps_xsingle"), "pe_pass_dr",
                                 sg_pass(mk, si, q)))

                # ACT: tent pair masks relu(1 - |vt2 - v1|)
                for p in range(NPAIR_DVE + NPAIR_GPS, NPAIR):
                    v1 = 2 * p + 1
                    q = ((v1 - 1) // BLK) * BLK
                    pa = p - (NPAIR_DVE + NPAIR_GPS)
                    ab = apool.tile([L, 2 * F], dt.bfloat16, tag="maskab")
                    nc.scalar.activation(
                        out=ab[:], in_=vt2[:], func=Act.Abs,
                        bias=abias_sb[:, pa:pa + 1], scale=1.0,
                    )
                    mp = apool.tile([L, 2 * F], dt.float8e4, tag="maskpa")
                    nc.scalar.activation(
                        out=mp[:], in_=ab[:], func=Act.Relu,
                        bias=1.0, scale=-1.0,
                    )
                    jobs.append((tick("act", "act_pair"), "pe_pass_dr",
                                 dr_pass(mp, p, q)))

                # ---- PE: S' first (starts the PSUM group), then greedy by
                # estimated mask-ready time ----
                for s in range(F // SUB):
                    sl = slice(s * SUB, (s + 1) * SUB)
                    nc.tensor.matmul(
                        out=pc[:, sl], lhsT=shi_sb[:], rhs=maskf[:, sl],
                        start=True, stop=False, skip_group_check=True,
                    )
                    nc.tensor.matmul(
                        out=pc[:, sl], lhsT=slo_sb[:], rhs=maskf[:, sl],
                        start=False, stop=False, skip_group_check=True,
                    )
                pe_t = maskf_ready + 2 * EST["pe_pass_bf16"]
                pending = sorted(jobs, key=lambda j: j[0])
                while pending:
                    ready = [j for j in pending if j[0] <= pe_t]
                    job = min(ready, key=lambda j: j[0]) if ready \
                        else pending[0]
                    pending.remove(job)
                    last = not pending
                    job[2](stop=last)
                    pe_t = max(pe_t, job[0]) + EST[job[1]]

                # D^T = PSUM + (prior' - C), recentred so bf16 is accurate
                dT = wpool.tile([64, F], dt.bfloat16, tag="dT")
                nc.scalar.activation(
                    out=dT[:], in_=pc[:, :], func=Act.Identity,
                    bias=prior_sb[:, 0:1], scale=1.0,
                )
                tick("act", "act_dT")

                # transpose 128-column tiles into one wide PSUM tile
                ptw = ptpool.tile([128, TPT * K], dt.bfloat16, tag="ptw")
                for t in range(TPT):
                    nc.tensor.transpose(
                        out=ptw[:, t * K:(t + 1) * K],
                        in_=dT[:, t * 128:(t + 1) * 128],
                        identity=identb[0:64, 0:64],
                    )
                cols = slice(c * TPT, (c + 1) * TPT)
                ptw3 = ptw[:].rearrange("p (t k) -> p t k", k=K)
                nc.vector.tensor_reduce(
                    out=mneg_all[:, cols], in_=ptw3,
                    axis=mybir.AxisListType.X, op=Alu.max, negate=True,
                )
                tick("dve", "dve_max")
                xm = tpool.tile([128, TPT * K], dt.bfloat16, tag="xm")
                ex = tpool.tile([128, TPT * K], dt.bfloat16, tag="ex")

                def tail(cols=cols, ptw3=ptw3, xm=xm, ex=ex):
                    # x - max on DVE (GPSIMD cannot read PSUM)
                    mneg_b = (mneg_all[:, cols].unsqueeze(-1)
                              .broadcast_to((128, TPT, K)))
                    nc.vector.tensor_tensor(
                        out=xm[:].rearrange("p (t k) -> p t k", k=K),
                        in0=ptw3, in1=mneg_b, op=Alu.add,
                    )
                    tick("dve", "dve_diff")
                    nc.scalar.activation(out=ex[:], in_=xm[:], func=Act.Exp)
                    tick("act", "act_exp")
                    nc.vector.tensor_reduce(
                        out=ssum_all[:, cols],
                        in_=ex[:].rearrange("p (t k) -> p t k", k=K),
                        axis=mybir.AxisListType.X, op=Alu.add,
                    )
                    tick("dve", "dve_sum")
                prev_tail = tail

            prev_tail()

            # finale: ln over all stashed sums, add back maxes + C, one DMA
            lns = cpool.tile([128, ncols], dt.float32, tag="lns")
            nc.scalar.activation(out=lns[:], in_=ssum_all[:], func=Act.Ln)
            outT = cpool.tile([128, ncols], dt.float32, tag="outT")
            nc.vector.scalar_tensor_tensor(
                out=outT[:], in0=lns[:], scalar=0.0, in1=mneg_all[:],
                op0=Alu.add, op1=Alu.subtract,
            )
            oview = out[:].rearrange("(x p) -> p x", p=128)
            nc.sync.dma_start(out=oview, in_=outT[:])
    nc.finalize()
    return nc


def _get_bass(nc_n: int):
    if nc_n not in _BASS_CACHE:
        _BASS_CACHE[nc_n] = _build_bass(nc_n)
    return _BASS_CACHE[nc_n]


def _prepare_host(votes, accuracy, propensity, class_balance):
    bf16 = ml_dtypes.bfloat16
    votes = np.asarray(votes)
    accuracy = np.asarray(accuracy, dtype=np.float32)
    propensity = np.asarray(propensity, dtype=np.float32)
    class_balance = np.asarray(class_balance, dtype=np.float32)

    # values 0..64 are exact in bf16
    votesT = np.ascontiguousarray(votes.T.astype(np.float32).astype(bf16))

    z_acc = np.logaddexp(accuracy, -accuracy)
    stab = (z_acc + accuracy - propensity[:, None] + LOGKM1).astype(np.float32)
    shi = stab.astype(bf16)
    slo = (stab - shi.astype(np.float32)).astype(bf16)
    nshi = np.ascontiguousarray(-shi)       # negated: PSUM accumulates -S'
    nslo = np.ascontiguousarray(-slo)

    w = 2.0 * accuracy + LOGKM1                      # [L, K]
    wblk = np.zeros((L, K, BLK), np.float32)
    jj = np.arange(K)
    wblk[:, jj, jj % BLK] = w                        # 32-wide block columns
    wblk = np.ascontiguousarray(wblk.reshape(L, K * BLK).astype(bf16))

    # fp8 DoubleRow pair weights (single fp8: corr terms are small)
    f8 = ml_dtypes.float8_e4m3
    wph = np.zeros((L, max(NPAIR, 1), 2, BLK), np.float32)
    w_f8 = w.astype(f8).astype(np.float32)
    for p in range(NPAIR):
        for i in range(2):
            j = 2 * p + i                            # target class row
            wph[:, p, i, j % BLK] = w_f8[:, j]
    wph = np.ascontiguousarray(
        wph.reshape(L, max(NPAIR, 1) * 2 * BLK).astype(f8))

    # byte-row DoubleRow weights for the bf16-X-mask singles: both byte
    # rows carry XBYTE*mask, so hi/lo-split w/XBYTE across the two rows.
    ws8 = np.zeros((L, NSING, 2, BLK), np.float32)
    for si in range(NSING):
        j = 2 * NPAIR + si                           # class row (v-1)
        a = w[:, j] / XBYTE
        w0 = a.astype(f8).astype(np.float32)
        w1 = (a - w0).astype(f8).astype(np.float32)
        ws8[:, si, 0, j % BLK] = w0
        ws8[:, si, 1, j % BLK] = w1
    ws8 = np.ascontiguousarray(ws8.reshape(L, NSING * 2 * BLK).astype(f8))

    zprop = np.logaddexp(propensity, 0.0)
    cbm = class_balance.max()
    cb = class_balance - (np.log(np.sum(np.exp(class_balance - cbm))) + cbm)
    priorp = (cb - zprop.sum()).astype(np.float32)

    # recentre so dT fits comfortably in bf16: C = exact x[j] max of inst 0
    v0 = votes[0].astype(np.int64)
    m0 = (v0 != 0).astype(np.float32)
    cll0 = (m0 @ propensity) - (m0 @ (z_acc + accuracy)) \
        - m0.sum() * LOGKM1
    idx0 = np.clip(v0 - 1, 0, K - 1)
    np.add.at(cll0, idx0, m0 * (2.0 * accuracy[np.arange(L), idx0] + LOGKM1))
    center = float((priorp + cll0).max())

    prior_c = np.ascontiguousarray((priorp - center).reshape(K, 1))

    abias = np.zeros((L, max(NPAIR_ACT, 1)), np.float32)
    for pa in range(NPAIR_ACT):
        abias[:, pa] = -float(2 * (NPAIR_DVE + NPAIR_GPS + pa) + 1)
    return votesT, wph, ws8, abias, nshi, nslo, prior_c, center


def _run(votes, accuracy, propensity, class_balance, trace=False):
    from concourse.bass_utils import run_bass_kernel_spmd

    votesT, wph, ws8, abias, nshi, nslo, prior_c, center = _prepare_host(
        votes, accuracy, propensity, class_balance
    )
    nc = _get_bass(NC_N)
    in_maps = []
    for c in range(M):
        in_maps.append({
            "votest": np.ascontiguousarray(votesT[:, c * NC_N:(c + 1) * NC_N]),
            "wph": wph,
            "ws8": ws8,
            "abias": abias,
            "nshi": nshi,
            "nslo": nslo,
            "prior": prior_c,
        })
    res = run_bass_kernel_spmd(
        nc, in_maps, core_ids=list(range(M)), trace=trace
    )
    out = np.concatenate([r["out"] for r in res.results])
    return (out + center).astype(np.float32), res


def kernel(votes, accuracy, propensity, class_balance):
    out, _ = _run(votes, accuracy, propensity, class_balance)
    return out


def kernel_with_stats(votes, accuracy, propensity, class_balance):
    try:
        out, res = _run(votes, accuracy, propensity, class_balance, trace=True)
    except (ImportError, ModuleNotFoundError):
        # no NTFF profiling hook in this environment; run without trace
        out, res = _run(votes, accuracy, propensity, class_balance, trace=False)
    return out, res


def simulate_ns() -> float:
    """Cost-model timeline estimate (ns) of one core's NEFF execution."""
    from concourse.timeline_sim import TimelineSim

    return TimelineSim(_get_bass(NC_N), trace=False).simulate()
